# revision 1
# baseline (speedup 1.0000x reference)
"""NetVLAD-style kernel for Trainium2, data-parallel over batch on 8 cores.

Reference computation (per sample b):
    f[k, n]   = sum_d w[k, d] * x[b, d, n]          (1x1 conv, N = H*W)
    a         = softmax over H of f (per (k, w) column)
    V[k, d]   = sum_n a[k, n] * x[b, d, n] - 32 * c[k, d]
                (a.sum(n) == W == 32 exactly, since softmax over H sums to 1)
    y[d, k]   = V[k, d] / ||V[:, d]||_2  (L2 norm over k; applied twice in the
                reference but idempotent)
Output: (D, K, B).

Device strategy (per core, 8 samples):
  - x loaded from HBM with SWDGE cast-DMA fp32->bf16 (natural (d, n) layout)
  - mm1: f = wT.T @ x on PE (bf16, fp32 PSUM accumulate)
  - exp on ScalarE (PSUM -> SBUF bf16), h-sum via strided DVE reduce,
    reciprocal on DVE, broadcast multiply -> a (bf16)
  - a transposed 128-col-chunk-wise on PE (identity matmul) -> aT (n, k)
  - x transposed via DMA xbar (bf16 SBUF->SBUF, 128x128 tiles) -> xT (n, d)
  - mm2: V^T = xT.T @ aT on PE, accumulated over n-chunks -> (d, k) PSUM
  - epilogue: subtract 32*c^T, square, reduce over k, rsqrt, scale -> y
  - one contiguous output DMA per d-chunk at the end
"""

import sys

for _p in ("/opt/trn_rl_repo", "/opt/trn_rl_repo/concourse"):
    if _p not in sys.path:
        sys.path.append(_p)

import ml_dtypes
import numpy as np

import concourse.bacc as bacc
import concourse.mybir as mybir
import concourse.tile as tile
from concourse.bass_utils import run_bass_kernel_spmd

B, D, H, W, K = 64, 512, 32, 32, 64
N = H * W          # 1024
M = 8              # cores
S = B // M         # samples per core
DC = D // 128      # 4 d-chunks
NCH = N // 128     # 8 n-chunks

F32 = mybir.dt.float32
BF16 = mybir.dt.bfloat16

_CACHE = {}


def _build_program():
    nc = bacc.Bacc(
        "TRN2",
        target_bir_lowering=False,
        debug=False,
        num_devices=M,
    )

    xs = nc.dram_tensor("xs", [S, D, N], F32, kind="ExternalInput").ap()
    wt = nc.dram_tensor("wt", [128, DC * K], BF16, kind="ExternalInput").ap()
    ct = nc.dram_tensor("ct", [128, DC * K], F32, kind="ExternalInput").ap()
    ident = nc.dram_tensor("ident", [K, K], BF16, kind="ExternalInput").ap()
    y = nc.dram_tensor("y", [D, K, S], F32, kind="ExternalOutput").ap()

    with tile.TileContext(nc) as tc:
        with (
            tc.tile_pool(name="consts", bufs=1) as cpool,
            tc.tile_pool(name="xin", bufs=2) as xpool,
            tc.tile_pool(name="xt", bufs=2) as xtpool,
            tc.tile_pool(name="soft", bufs=2) as spool,
            tc.tile_pool(name="at", bufs=2) as atpool,
            tc.tile_pool(name="epi", bufs=2) as epool,
            tc.tile_pool(name="fpsum", bufs=2, space="PSUM") as fpool,
            tc.tile_pool(name="atpsum", bufs=2, space="PSUM") as atppool,
            tc.tile_pool(name="vpsum", bufs=2, space="PSUM") as vpool,
        ):
            wt_sb = cpool.tile([128, DC * K], BF16)
            nc.sync.dma_start(wt_sb[:], wt)
            ct_sb = cpool.tile([128, DC * K], F32)
            nc.sync.dma_start(ct_sb[:], ct)
            id_sb = cpool.tile([K, K], BF16)
            nc.sync.dma_start(id_sb[:], ident)
            # y accumulator: free layout dc*512 + k*8 + s
            y_all = cpool.tile([128, DC * K * S], F32)

            for s in range(S):
                # --- load x (cast fp32 -> bf16), natural (d, n) layout ---
                x_sb = xpool.tile([128, DC * N], BF16, tag="x")
                for dc in range(DC):
                    nc.gpsimd.dma_start(
                        x_sb[:, dc * N:(dc + 1) * N],
                        xs[s, dc * 128:(dc + 1) * 128, :],
                    )

                # --- mm1: f[k, n] ---
                f_ps = fpool.tile([K, N], F32, tag="f")
                for dc in range(DC):
                    for hh in range(2):
                        nc.tensor.matmul(
                            f_ps[:, hh * 512:(hh + 1) * 512],
                            lhsT=wt_sb[:, dc * K:(dc + 1) * K],
                            rhs=x_sb[:, dc * N + hh * 512: dc * N + (hh + 1) * 512],
                            start=(dc == 0),
                            stop=(dc == DC - 1),
                        )

                # --- softmax over h (n = h*32 + w) ---
                e_sb = spool.tile([K, N], BF16, tag="e")
                nc.scalar.activation(
                    e_sb[:], f_ps[:], mybir.ActivationFunctionType.Exp
                )
                ssum = spool.tile([K, W], F32, tag="ssum")
                nc.vector.reduce_sum(
                    ssum[:],
                    e_sb.rearrange("k (h w) -> k w h", h=H, w=W),
                    axis=mybir.AxisListType.X,
                )
                rinv = spool.tile([K, W], F32, tag="rinv")
                nc.vector.reciprocal(rinv[:], ssum[:])
                a_sb = spool.tile([K, N], BF16, tag="a")
                nc.vector.tensor_mul(
                    a_sb.rearrange("k (h w) -> k h w", h=H, w=W),
                    e_sb.rearrange("k (h w) -> k h w", h=H, w=W),
                    rinv.unsqueeze(1).broadcast_to([K, H, W]),
                )

                # --- aT via PE transpose, chunk-wise ---
                at_sb = atpool.tile([128, NCH * K], BF16, tag="at")
                for nck in range(NCH):
                    at_ps = atppool.tile([128, K], BF16, tag="atp")
                    nc.tensor.transpose(
                        at_ps[:], a_sb[:, nck * 128:(nck + 1) * 128], id_sb[:]
                    )
                    nc.vector.tensor_copy(
                        at_sb[:, nck * K:(nck + 1) * K], at_ps[:]
                    )

                # --- xT via DMA xbar transpose (bf16) ---
                xt_sb = xtpool.tile([128, NCH * D], BF16, tag="xt")
                for nck in range(NCH):
                    for dc in range(DC):
                        nc.sync.dma_start(
                            xt_sb[:, nck * D + dc * 128: nck * D + (dc + 1) * 128],
                            x_sb[:, dc * N + nck * 128: dc * N + (nck + 1) * 128],
                            transpose=True,
                        )

                # --- mm2: V^T[d, k] accumulated over n-chunks ---
                v_ps = vpool.tile([128, DC * K], F32, tag="v")
                for dc in range(DC):
                    for nck in range(NCH):
                        nc.tensor.matmul(
                            v_ps[:, dc * K:(dc + 1) * K],
                            lhsT=xt_sb[:, nck * D + dc * 128: nck * D + (dc + 1) * 128],
                            rhs=at_sb[:, nck * K:(nck + 1) * K],
                            start=(nck == 0),
                            stop=(nck == NCH - 1),
                        )

                # --- epilogue: V - 32*c, normalize over k ---
                vc = epool.tile([128, DC * K], F32, tag="vc")
                nc.vector.tensor_sub(vc[:], v_ps[:], ct_sb[:])
                sq = epool.tile([128, DC * K], F32, tag="sq")
                nc.scalar.activation(
                    sq[:], vc[:], mybir.ActivationFunctionType.Square
                )
                nsq = epool.tile([128, DC], F32, tag="nsq")
                nc.vector.reduce_sum(
                    nsq[:],
                    sq.rearrange("p (c k) -> p c k", c=DC, k=K),
                    axis=mybir.AxisListType.X,
                )
                rn = epool.tile([128, DC], F32, tag="rn")
                nc.vector.reciprocal(rn[:], nsq[:])
                rs = epool.tile([128, DC], F32, tag="rs")
                nc.scalar.sqrt(rs[:], rn[:])
                y_view = y_all.rearrange("p (c k s) -> p c k s", c=DC, k=K, s=S)
                for dc in range(DC):
                    nc.scalar.activation(
                        y_view[:, dc, :, s],
                        vc[:, dc * K:(dc + 1) * K],
                        mybir.ActivationFunctionType.Copy,
                        scale=rs[:, dc:dc + 1],
                    )

            # --- output: one contiguous DMA per d-chunk ---
            y_out = y.rearrange("(c p) k s -> c p (k s)", c=DC, p=128)
            for dc in range(DC):
                nc.sync.dma_start(
                    y_out[dc], y_all[:, dc * K * S:(dc + 1) * K * S]
                )

    nc.compile()
    return nc


def _get_program():
    if "nc" not in _CACHE:
        _CACHE["nc"] = _build_program()
    return _CACHE["nc"]


def kernel(x: np.ndarray, w: np.ndarray, c: np.ndarray) -> np.ndarray:
    nc = _get_program()

    xs_full = np.ascontiguousarray(x.reshape(B, D, N), dtype=np.float32)
    wt_host = np.ascontiguousarray(
        w.T.reshape(DC, 128, K).transpose(1, 0, 2).reshape(128, DC * K)
    ).astype(ml_dtypes.bfloat16)
    ct_host = np.ascontiguousarray(
        (32.0 * c).T.reshape(DC, 128, K).transpose(1, 0, 2).reshape(128, DC * K)
    ).astype(np.float32)
    id_host = np.eye(K, dtype=ml_dtypes.bfloat16)

    in_maps = []
    for m in range(M):
        in_maps.append(
            {
                "xs": xs_full[m * S:(m + 1) * S],
                "wt": wt_host,
                "ct": ct_host,
                "ident": id_host,
            }
        )

    res = run_bass_kernel_spmd(nc, in_maps, core_ids=list(range(M)))
    y = np.concatenate([res.results[m]["y"] for m in range(M)], axis=2)
    return y.astype(np.float32)


if __name__ == "__main__":
    rng = np.random.default_rng(0)
    x = rng.standard_normal((B, D, H, W), dtype=np.float32)
    w = (rng.standard_normal((K, D)) * 0.05).astype(np.float32)
    c = rng.standard_normal((K, D), dtype=np.float32)
    out = kernel(x=x, w=w, c=c)
    print(out.shape, out.dtype)


# revision 2
# speedup vs baseline: 4.3925x; 4.3925x over previous
"""NetVLAD-style kernel for Trainium2, data-parallel over batch on 8 cores.

Reference computation (per sample b):
    f[k, n]   = sum_d w[k, d] * x[b, d, n]          (1x1 conv, N = H*W)
    a         = softmax over H of f (per (k, w) column)
    V[k, d]   = sum_n a[k, n] * x[b, d, n] - 32 * c[k, d]
                (a.sum(n) == W == 32 exactly: softmax over H sums to 1)
    y[d, k]   = V[k, d] / ||V[:, d]||_2  (L2 over k; applied twice in the
                reference but idempotent)
Output: (D, K, B).

Device strategy (per core, 8 samples), "orientation B":
  - x loaded from HBM with SWDGE cast-DMA fp32->bf16, natural (d, n) layout
  - per 128x128 chunk of x, the chunk is the PE stationary operand for BOTH:
      * fT[n, k] += x_chunk.T @ wT_chunk   (fp32 PSUM, accum over d-chunks)
      * xT block = x_chunk.T @ I_128       (PE transpose, bf16 PSUM)
    so fT and a land with n on partitions and xT needs no DMA-xbar/strided
    transpose at all.
  - exp on ScalarE (PSUM -> SBUF bf16) on full 128 partitions
  - softmax denominators: sT[w, k] = sum_n Hmask[n, w] * eT[n, k] via one
    Hmask-stationary PE matmul chain; reciprocal on DVE; replicated across
    partition groups by 4 tiny DMAs; aT = eT * r in one DVE multiply
  - mm2: V^T[d, k] = sum_n xT[n, d]^T aT[n, k] accumulated over n-chunks
  - epilogue: subtract 32*c^T, square, segment-reduce over k, rsqrt, scale
  - one contiguous output DMA per d-chunk at the end
"""

import sys

for _p in ("/opt/trn_rl_repo", "/opt/trn_rl_repo/concourse"):
    if _p not in sys.path:
        sys.path.append(_p)

import ml_dtypes
import numpy as np

import concourse.bacc as bacc
import concourse.mybir as mybir
import concourse.tile as tile
from concourse.bass_utils import run_bass_kernel_spmd

B, D, H, W, K = 64, 512, 32, 32, 64
N = H * W          # 1024
M = 8              # cores
S = B // M         # samples per core
DC = D // 128      # 4 d-chunks
NCH = N // 128     # 8 n-chunks

F32 = mybir.dt.float32
BF16 = mybir.dt.bfloat16

_CACHE = {}


def _build_program():
    nc = bacc.Bacc(
        "TRN2",
        target_bir_lowering=False,
        debug=False,
        num_devices=M,
    )

    xs = nc.dram_tensor("xs", [S, D, N], F32, kind="ExternalInput").ap()
    wt = nc.dram_tensor("wt", [128, DC * K], BF16, kind="ExternalInput").ap()
    ct = nc.dram_tensor("ct", [128, DC * K], F32, kind="ExternalInput").ap()
    id128 = nc.dram_tensor("id128", [128, 128], BF16, kind="ExternalInput").ap()
    hmask = nc.dram_tensor("hmask", [128, W], BF16, kind="ExternalInput").ap()
    y = nc.dram_tensor("y", [D, K, S], F32, kind="ExternalOutput").ap()

    with tile.TileContext(nc) as tc:
        with (
            tc.tile_pool(name="consts", bufs=1) as cpool,
            tc.tile_pool(name="xin", bufs=2) as xpool,
            tc.tile_pool(name="xt", bufs=2) as xtpool,
            tc.tile_pool(name="soft", bufs=2) as spool,
            tc.tile_pool(name="epi", bufs=2) as epool,
            tc.tile_pool(name="fpsum", bufs=2, space="PSUM") as fpool,
            tc.tile_pool(name="xtpsum", bufs=3, space="PSUM") as xtppool,
            tc.tile_pool(name="spsum", bufs=1, space="PSUM") as sppool,
            tc.tile_pool(name="vpsum", bufs=2, space="PSUM") as vpool,
        ):
            wt_sb = cpool.tile([128, DC * K], BF16)
            nc.sync.dma_start(wt_sb[:], wt)
            ct_sb = cpool.tile([128, DC * K], F32)
            nc.sync.dma_start(ct_sb[:], ct)
            id_sb = cpool.tile([128, 128], BF16)
            nc.sync.dma_start(id_sb[:], id128)
            hm_sb = cpool.tile([128, W], BF16)
            nc.sync.dma_start(hm_sb[:], hmask)
            # y accumulator: free layout dc*512 + k*8 + s
            y_all = cpool.tile([128, DC * K * S], F32)

            for s in range(S):
                # --- load x (cast fp32 -> bf16), natural (d, n) layout ---
                x_sb = xpool.tile([128, DC * N], BF16, tag="x")
                for dc in range(DC):
                    nc.gpsimd.dma_start(
                        x_sb[:, dc * N:(dc + 1) * N],
                        xs[s, dc * 128:(dc + 1) * 128, :],
                    )

                # --- mm1 (fT) + PE transpose of x, sharing the stationary ---
                fT_ps = fpool.tile([128, NCH * K], F32, tag="f")
                xt_sb = xtpool.tile([128, NCH * D], BF16, tag="xt")
                for nck in range(NCH):
                    xt_ps = xtppool.tile([128, D], BF16, tag="xtp")
                    for dc in range(DC):
                        x_chunk = x_sb[:, dc * N + nck * 128: dc * N + (nck + 1) * 128]
                        nc.tensor.matmul(
                            fT_ps[:, nck * K:(nck + 1) * K],
                            lhsT=x_chunk,
                            rhs=wt_sb[:, dc * K:(dc + 1) * K],
                            start=(dc == 0),
                            stop=(dc == DC - 1),
                        )
                        nc.tensor.transpose(
                            xt_ps[:, dc * 128:(dc + 1) * 128], x_chunk, id_sb[:]
                        )
                    # copy xT chunk PSUM -> SBUF (alternate DVE / ACT)
                    dst = xt_sb[:, nck * D:(nck + 1) * D]
                    if nck % 2 == 0:
                        nc.vector.tensor_copy(dst, xt_ps[:])
                    else:
                        nc.scalar.copy(dst, xt_ps[:])

                # --- softmax over h (partitions hold n = h*32 + w) ---
                eT_sb = spool.tile([128, NCH * K], BF16, tag="e")
                nc.scalar.activation(
                    eT_sb[:], fT_ps[:], mybir.ActivationFunctionType.Exp
                )
                sT_ps = sppool.tile([W, K], F32, tag="sT")
                for nck in range(NCH):
                    nc.tensor.matmul(
                        sT_ps[:],
                        lhsT=hm_sb[:],
                        rhs=eT_sb[:, nck * K:(nck + 1) * K],
                        start=(nck == 0),
                        stop=(nck == NCH - 1),
                    )
                rT_sb = spool.tile([W, K], F32, tag="rT")
                nc.vector.reciprocal(rT_sb[:], sT_ps[:])
                r_tile = spool.tile([128, K], F32, tag="rtile")
                for g in range(4):
                    nc.gpsimd.dma_start(r_tile[g * W:(g + 1) * W, :], rT_sb[:])
                aT_sb = spool.tile([128, NCH * K], BF16, tag="a")
                nc.vector.tensor_mul(
                    aT_sb.rearrange("p (c k) -> p c k", c=NCH, k=K),
                    eT_sb.rearrange("p (c k) -> p c k", c=NCH, k=K),
                    r_tile.unsqueeze(1).broadcast_to([128, NCH, K]),
                )

                # --- mm2: V^T[d, k] accumulated over n-chunks ---
                v_ps = vpool.tile([128, DC * K], F32, tag="v")
                for dc in range(DC):
                    for nck in range(NCH):
                        nc.tensor.matmul(
                            v_ps[:, dc * K:(dc + 1) * K],
                            lhsT=xt_sb[:, nck * D + dc * 128: nck * D + (dc + 1) * 128],
                            rhs=aT_sb[:, nck * K:(nck + 1) * K],
                            start=(nck == 0),
                            stop=(nck == NCH - 1),
                        )

                # --- epilogue: V - 32*c, normalize over k ---
                vc = epool.tile([128, DC * K], F32, tag="vc")
                nc.vector.tensor_sub(vc[:], v_ps[:], ct_sb[:])
                sq = epool.tile([128, DC * K], F32, tag="sq")
                nc.scalar.activation(
                    sq[:], vc[:], mybir.ActivationFunctionType.Square
                )
                nsq = epool.tile([128, DC], F32, tag="nsq")
                nc.vector.reduce_sum(
                    nsq[:],
                    sq.rearrange("p (c k) -> p c k", c=DC, k=K),
                    axis=mybir.AxisListType.X,
                )
                rn = epool.tile([128, DC], F32, tag="rn")
                nc.vector.reciprocal(rn[:], nsq[:])
                rs = epool.tile([128, DC], F32, tag="rs")
                nc.scalar.sqrt(rs[:], rn[:])
                y_view = y_all.rearrange("p (c k s) -> p c k s", c=DC, k=K, s=S)
                for dc in range(DC):
                    nc.scalar.activation(
                        y_view[:, dc, :, s],
                        vc[:, dc * K:(dc + 1) * K],
                        mybir.ActivationFunctionType.Copy,
                        scale=rs[:, dc:dc + 1],
                    )

            # --- output: one contiguous DMA per d-chunk ---
            y_out = y.rearrange("(c p) k s -> c p (k s)", c=DC, p=128)
            for dc in range(DC):
                nc.sync.dma_start(
                    y_out[dc], y_all[:, dc * K * S:(dc + 1) * K * S]
                )

    nc.compile()
    return nc


def _get_program():
    if "nc" not in _CACHE:
        _CACHE["nc"] = _build_program()
    return _CACHE["nc"]


def _host_inputs(w: np.ndarray, c: np.ndarray):
    wt_host = np.ascontiguousarray(
        w.T.reshape(DC, 128, K).transpose(1, 0, 2).reshape(128, DC * K)
    ).astype(ml_dtypes.bfloat16)
    ct_host = np.ascontiguousarray(
        (32.0 * c).T.reshape(DC, 128, K).transpose(1, 0, 2).reshape(128, DC * K)
    ).astype(np.float32)
    id_host = np.eye(128, dtype=ml_dtypes.bfloat16)
    hm_host = np.zeros((128, W), dtype=ml_dtypes.bfloat16)
    for n in range(128):
        hm_host[n, n % W] = 1.0
    return wt_host, ct_host, id_host, hm_host


def kernel(x: np.ndarray, w: np.ndarray, c: np.ndarray) -> np.ndarray:
    nc = _get_program()

    xs_full = np.ascontiguousarray(x.reshape(B, D, N), dtype=np.float32)
    wt_host, ct_host, id_host, hm_host = _host_inputs(w, c)

    in_maps = []
    for m in range(M):
        in_maps.append(
            {
                "xs": xs_full[m * S:(m + 1) * S],
                "wt": wt_host,
                "ct": ct_host,
                "id128": id_host,
                "hmask": hm_host,
            }
        )

    res = run_bass_kernel_spmd(nc, in_maps, core_ids=list(range(M)))
    y = np.concatenate([res.results[m]["y"] for m in range(M)], axis=2)
    return y.astype(np.float32)


if __name__ == "__main__":
    rng = np.random.default_rng(0)
    x = rng.standard_normal((B, D, H, W), dtype=np.float32)
    w = (rng.standard_normal((K, D)) * 0.05).astype(np.float32)
    c = rng.standard_normal((K, D), dtype=np.float32)
    out = kernel(x=x, w=w, c=c)
    print(out.shape, out.dtype)


# revision 5
# speedup vs baseline: 5.0581x; 1.1515x over previous
"""NetVLAD-style kernel for Trainium2, data-parallel over batch on 8 cores.

Reference computation (per sample b):
    f[k, n]   = sum_d w[k, d] * x[b, d, n]          (1x1 conv, N = H*W)
    a         = softmax over H of f (per (k, w) column)
    V[k, d]   = sum_n a[k, n] * x[b, d, n] - 32 * c[k, d]
                (a.sum(n) == W == 32 exactly: softmax over H sums to 1)
    y[d, k]   = V[k, d] / ||V[:, d]||_2  (L2 over k; applied twice in the
                reference but idempotent)
Output: (D, K, B).

Device strategy (per core, 8 samples), "orientation B":
  - x loaded from HBM with SWDGE cast-DMA fp32->bf16, natural (d, n) layout
  - per 128x128 chunk of x, the chunk is the PE stationary operand for BOTH:
      * fT[n, k] += x_chunk.T @ wT_chunk   (fp32 PSUM, accum over d-chunks)
      * xT block = x_chunk.T @ I_128       (PE transpose, bf16 PSUM)
    so fT and a land with n on partitions and xT needs no DMA-xbar/strided
    transpose at all.
  - exp on ScalarE (PSUM -> SBUF bf16) on full 128 partitions
  - softmax denominators: sT[w, k] = sum_n Hmask[n, w] * eT[n, k] via one
    Hmask-stationary PE matmul chain; reciprocal on DVE; replicated across
    partition groups by 4 tiny DMAs; aT = eT * r in one DVE multiply
  - mm2: V^T[d, k] = sum_n xT[n, d]^T aT[n, k] accumulated over n-chunks
  - epilogue: subtract 32*c^T, square, segment-reduce over k, rsqrt, scale
  - one contiguous output DMA per d-chunk at the end
"""

import sys

for _p in ("/opt/trn_rl_repo", "/opt/trn_rl_repo/concourse"):
    if _p not in sys.path:
        sys.path.append(_p)

import ml_dtypes
import numpy as np

import concourse.bacc as bacc
import concourse.mybir as mybir
import concourse.tile as tile
from concourse.bass_utils import run_bass_kernel_spmd

B, D, H, W, K = 64, 512, 32, 32, 64
N = H * W          # 1024
M = 8              # cores
S = B // M         # samples per core
DC = D // 128      # 4 d-chunks
NCH = N // 128     # 8 n-chunks

F32 = mybir.dt.float32
BF16 = mybir.dt.bfloat16

_CACHE = {}


def _build_program():
    nc = bacc.Bacc(
        "TRN2",
        target_bir_lowering=False,
        debug=False,
        num_devices=M,
    )

    xs = nc.dram_tensor("xs", [S, D, N], F32, kind="ExternalInput").ap()
    wt = nc.dram_tensor("wt", [128, DC * K], BF16, kind="ExternalInput").ap()
    ct = nc.dram_tensor("ct", [128, DC * K], F32, kind="ExternalInput").ap()
    id128 = nc.dram_tensor("id128", [128, 128], BF16, kind="ExternalInput").ap()
    hmask = nc.dram_tensor("hmask", [128, W], BF16, kind="ExternalInput").ap()
    y = nc.dram_tensor("y", [D, K, S], F32, kind="ExternalOutput").ap()

    with tile.TileContext(nc) as tc:
        with (
            tc.tile_pool(name="consts", bufs=1) as cpool,
            tc.tile_pool(name="xin", bufs=2) as xpool,
            tc.tile_pool(name="xt", bufs=2) as xtpool,
            tc.tile_pool(name="soft", bufs=2) as spool,
            tc.tile_pool(name="epi", bufs=2) as epool,
            tc.tile_pool(name="fpsum", bufs=2, space="PSUM") as fpool,
            tc.tile_pool(name="xtpsum", bufs=3, space="PSUM") as xtppool,
            tc.tile_pool(name="spsum", bufs=1, space="PSUM") as sppool,
            tc.tile_pool(name="vpsum", bufs=2, space="PSUM") as vpool,
        ):
            wt_sb = cpool.tile([128, DC * K], BF16)
            nc.sync.dma_start(wt_sb[:], wt)
            ct_sb = cpool.tile([128, DC * K], F32)
            nc.sync.dma_start(ct_sb[:], ct)
            id_sb = cpool.tile([128, 128], BF16)
            nc.sync.dma_start(id_sb[:], id128)
            hm_sb = cpool.tile([128, W], BF16)
            nc.sync.dma_start(hm_sb[:], hmask)
            # y accumulator: free layout dc*512 + k*8 + s
            y_all = cpool.tile([128, DC * K * S], F32)
            # per-sample V - 32c staging and batched norm^2 (epilogue is
            # batched across samples so ACT does one table switch, not 8)
            vc_all = cpool.tile([128, S * DC * K], F32)
            nsq_all = cpool.tile([128, S * DC], F32)

            for s in range(S):
                # --- load x (cast fp32 -> bf16), natural (d, n) layout ---
                x_sb = xpool.tile([128, DC * N], BF16, tag="x")
                for dc in range(DC):
                    nc.gpsimd.dma_start(
                        x_sb[:, dc * N:(dc + 1) * N],
                        xs[s, dc * 128:(dc + 1) * 128, :],
                    )

                # --- mm1 (fT) + PE transpose of x, sharing the stationary ---
                fT_ps = fpool.tile([128, NCH * K], F32, tag="f")
                xt_sb = xtpool.tile([128, NCH * D], BF16, tag="xt")
                for nck in range(NCH):
                    xt_ps = xtppool.tile([128, D], BF16, tag="xtp")
                    for dc in range(DC):
                        x_chunk = x_sb[:, dc * N + nck * 128: dc * N + (nck + 1) * 128]
                        nc.tensor.matmul(
                            fT_ps[:, nck * K:(nck + 1) * K],
                            lhsT=x_chunk,
                            rhs=wt_sb[:, dc * K:(dc + 1) * K],
                            start=(dc == 0),
                            stop=(dc == DC - 1),
                        )
                        nc.tensor.transpose(
                            xt_ps[:, dc * 128:(dc + 1) * 128], x_chunk, id_sb[:]
                        )
                    # copy xT chunk PSUM -> SBUF (alternate DVE / ACT)
                    dst = xt_sb[:, nck * D:(nck + 1) * D]
                    if nck % 2 == 0:
                        nc.vector.tensor_copy(dst, xt_ps[:])
                    else:
                        nc.scalar.copy(dst, xt_ps[:])

                # --- softmax over h (partitions hold n = h*32 + w) ---
                eT_sb = spool.tile([128, NCH * K], BF16, tag="e")
                nc.scalar.activation(
                    eT_sb[:], fT_ps[:], mybir.ActivationFunctionType.Exp
                )
                sT_ps = sppool.tile([W, K], F32, tag="sT")
                for nck in range(NCH):
                    nc.tensor.matmul(
                        sT_ps[:],
                        lhsT=hm_sb[:],
                        rhs=eT_sb[:, nck * K:(nck + 1) * K],
                        start=(nck == 0),
                        stop=(nck == NCH - 1),
                    )
                rT_sb = spool.tile([W, K], F32, tag="rT")
                nc.vector.reciprocal(rT_sb[:], sT_ps[:])
                r_tile = spool.tile([128, K], F32, tag="rtile")
                for g in range(4):
                    nc.sync.dma_start(r_tile[g * W:(g + 1) * W, :], rT_sb[:])
                aT_sb = spool.tile([128, NCH * K], BF16, tag="a")
                nc.vector.tensor_mul(
                    aT_sb.rearrange("p (c k) -> p c k", c=NCH, k=K),
                    eT_sb.rearrange("p (c k) -> p c k", c=NCH, k=K),
                    r_tile.unsqueeze(1).broadcast_to([128, NCH, K]),
                )

                # --- mm2: V^T[d, k] accumulated over n-chunks ---
                v_ps = vpool.tile([128, DC * K], F32, tag="v")
                for dc in range(DC):
                    for nck in range(NCH):
                        nc.tensor.matmul(
                            v_ps[:, dc * K:(dc + 1) * K],
                            lhsT=xt_sb[:, nck * D + dc * 128: nck * D + (dc + 1) * 128],
                            rhs=aT_sb[:, nck * K:(nck + 1) * K],
                            start=(nck == 0),
                            stop=(nck == NCH - 1),
                        )

                # --- per-sample epilogue part: V - 32*c, squared k-sums ---
                vc = vc_all[:, s * DC * K:(s + 1) * DC * K]
                nc.vector.tensor_sub(vc, v_ps[:], ct_sb[:])
                sq = epool.tile([128, DC * K], F32, tag="sq")
                nc.vector.tensor_mul(sq[:], vc, vc)
                nc.vector.reduce_sum(
                    nsq_all[:, s * DC:(s + 1) * DC],
                    sq.rearrange("p (c k) -> p c k", c=DC, k=K),
                    axis=mybir.AxisListType.X,
                )

            # --- batched normalization tail ---
            rn_all = cpool.tile([128, S * DC], F32)
            nc.vector.reciprocal(rn_all[:], nsq_all[:])
            rs_all = cpool.tile([128, S * DC], F32)
            nc.scalar.sqrt(rs_all[:], rn_all[:])
            y_view = y_all.rearrange("p (c k s) -> p c k s", c=DC, k=K, s=S)
            for s in range(S):
                for dc in range(DC):
                    nc.scalar.activation(
                        y_view[:, dc, :, s],
                        vc_all[:, (s * DC + dc) * K:(s * DC + dc + 1) * K],
                        mybir.ActivationFunctionType.Copy,
                        scale=rs_all[:, s * DC + dc:s * DC + dc + 1],
                    )

            # --- output: one contiguous DMA per d-chunk ---
            y_out = y.rearrange("(c p) k s -> c p (k s)", c=DC, p=128)
            for dc in range(DC):
                nc.sync.dma_start(
                    y_out[dc], y_all[:, dc * K * S:(dc + 1) * K * S]
                )

    nc.compile()
    return nc


def _get_program():
    if "nc" not in _CACHE:
        _CACHE["nc"] = _build_program()
    return _CACHE["nc"]


def _host_inputs(w: np.ndarray, c: np.ndarray):
    wt_host = np.ascontiguousarray(
        w.T.reshape(DC, 128, K).transpose(1, 0, 2).reshape(128, DC * K)
    ).astype(ml_dtypes.bfloat16)
    ct_host = np.ascontiguousarray(
        (32.0 * c).T.reshape(DC, 128, K).transpose(1, 0, 2).reshape(128, DC * K)
    ).astype(np.float32)
    id_host = np.eye(128, dtype=ml_dtypes.bfloat16)
    hm_host = np.zeros((128, W), dtype=ml_dtypes.bfloat16)
    for n in range(128):
        hm_host[n, n % W] = 1.0
    return wt_host, ct_host, id_host, hm_host


def kernel(x: np.ndarray, w: np.ndarray, c: np.ndarray) -> np.ndarray:
    nc = _get_program()

    xs_full = np.ascontiguousarray(x.reshape(B, D, N), dtype=np.float32)
    wt_host, ct_host, id_host, hm_host = _host_inputs(w, c)

    in_maps = []
    for m in range(M):
        in_maps.append(
            {
                "xs": xs_full[m * S:(m + 1) * S],
                "wt": wt_host,
                "ct": ct_host,
                "id128": id_host,
                "hmask": hm_host,
            }
        )

    res = run_bass_kernel_spmd(nc, in_maps, core_ids=list(range(M)))
    y = np.concatenate([res.results[m]["y"] for m in range(M)], axis=2)
    return y.astype(np.float32)


if __name__ == "__main__":
    rng = np.random.default_rng(0)
    x = rng.standard_normal((B, D, H, W), dtype=np.float32)
    w = (rng.standard_normal((K, D)) * 0.05).astype(np.float32)
    c = rng.standard_normal((K, D), dtype=np.float32)
    out = kernel(x=x, w=w, c=c)
    print(out.shape, out.dtype)


# revision 12
# speedup vs baseline: 5.2160x; 1.0312x over previous
"""NetVLAD-style kernel for Trainium2, data-parallel over batch on 8 cores.

Reference computation (per sample b):
    f[k, n]   = sum_d w[k, d] * x[b, d, n]          (1x1 conv, N = H*W)
    a         = softmax over H of f (per (k, w) column)
    V[k, d]   = sum_n a[k, n] * x[b, d, n] - 32 * c[k, d]
                (a.sum(n) == W == 32 exactly: softmax over H sums to 1)
    y[d, k]   = V[k, d] / ||V[:, d]||_2  (L2 over k; applied twice in the
                reference but idempotent)
Output: (D, K, B).

Device strategy (per core, 8 samples), "orientation B":
  - x loaded from HBM with SWDGE cast-DMA fp32->bf16, natural (d, n) layout
  - per 128x128 chunk of x, the chunk is the PE stationary operand for BOTH:
      * fT[n, k] += x_chunk.T @ wT_chunk   (fp32 PSUM, accum over d-chunks)
      * xT block = x_chunk.T @ I_128       (PE transpose, bf16 PSUM)
    so fT and a land with n on partitions and xT needs no DMA-xbar/strided
    transpose at all.
  - exp on ScalarE (PSUM -> SBUF bf16) on full 128 partitions
  - softmax denominators: sT[w, k] = sum_n Hmask[n, w] * eT[n, k] via one
    Hmask-stationary PE matmul chain; reciprocal on DVE; replicated across
    partition groups by 4 tiny DMAs; aT = eT * r in one DVE multiply
  - mm2: V^T[d, k] = sum_n xT[n, d]^T aT[n, k] accumulated over n-chunks
  - epilogue: subtract 32*c^T, square, segment-reduce over k, rsqrt, scale
  - one contiguous output DMA per d-chunk at the end
"""

import sys

for _p in ("/opt/trn_rl_repo", "/opt/trn_rl_repo/concourse"):
    if _p not in sys.path:
        sys.path.append(_p)

import ml_dtypes
import numpy as np

import concourse.bacc as bacc
import concourse.mybir as mybir
import concourse.tile as tile
from concourse.bass_utils import run_bass_kernel_spmd

B, D, H, W, K = 64, 512, 32, 32, 64
N = H * W          # 1024
M = 8              # cores
S = B // M         # samples per core
DC = D // 128      # 4 d-chunks
NCH = N // 128     # 8 n-chunks

F32 = mybir.dt.float32
BF16 = mybir.dt.bfloat16

_CACHE = {}


def _build_program():
    nc = bacc.Bacc(
        "TRN2",
        target_bir_lowering=False,
        debug=False,
        num_devices=M,
    )

    xs = nc.dram_tensor("xs", [S, D, N], F32, kind="ExternalInput").ap()
    wt = nc.dram_tensor("wt", [128, DC * K], BF16, kind="ExternalInput").ap()
    ct = nc.dram_tensor("ct", [128, DC * K], F32, kind="ExternalInput").ap()
    id128 = nc.dram_tensor("id128", [128, 128], BF16, kind="ExternalInput").ap()
    # hmask4[n, g*W + w] = (n % W == w), g = 0..3: the sT matmul output then
    # lands pre-replicated across the 4 partition groups -> usable directly
    # as the broadcast multiplier tile.
    hmask = nc.dram_tensor("hmask", [128, 4 * W], BF16, kind="ExternalInput").ap()
    y = nc.dram_tensor("y", [D, K, S], F32, kind="ExternalOutput").ap()

    with tile.TileContext(nc) as tc:
        with (
            tc.tile_pool(name="consts", bufs=1) as cpool,
            tc.tile_pool(name="xin", bufs=2) as xpool,
            tc.tile_pool(name="xt", bufs=2) as xtpool,
            tc.tile_pool(name="soft", bufs=2) as spool,
            tc.tile_pool(name="epi", bufs=2) as epool,
            tc.tile_pool(name="fpsum", bufs=3, space="PSUM") as fpool,
            tc.tile_pool(name="xtpsum", bufs=2, space="PSUM") as xtppool,
            tc.tile_pool(name="spsum", bufs=1, space="PSUM") as sppool,
            tc.tile_pool(name="vpsum", bufs=2, space="PSUM") as vpool,
        ):
            wt_sb = cpool.tile([128, DC * K], BF16)
            nc.sync.dma_start(wt_sb[:], wt)
            ct_sb = cpool.tile([128, DC * K], F32)
            nc.sync.dma_start(ct_sb[:], ct)
            id_sb = cpool.tile([128, 128], BF16)
            nc.sync.dma_start(id_sb[:], id128)
            hm_sb = cpool.tile([128, 4 * W], BF16)
            nc.sync.dma_start(hm_sb[:], hmask)
            # y accumulator: free layout dc*512 + k*8 + s
            y_all = cpool.tile([128, DC * K * S], F32)
            # per-sample V - 32c staging and batched norm^2 (epilogue is
            # batched across samples so ACT does one table switch, not 8)
            vc_all = cpool.tile([128, S * DC * K], F32)
            nsq_all = cpool.tile([128, S * DC], F32)

            for s in range(S):
                # --- load x (cast fp32 -> bf16), natural (d, n) layout ---
                x_sb = xpool.tile([128, DC * N], BF16, tag="x")
                for dc in range(DC):
                    nc.gpsimd.dma_start(
                        x_sb[:, dc * N:(dc + 1) * N],
                        xs[s, dc * 128:(dc + 1) * 128, :],
                    )

                # --- mm1 (fT) + PE transpose of x, sharing the stationary ---
                fT_ps = fpool.tile([128, NCH * K], F32, tag="f")
                xt_sb = xtpool.tile([128, NCH * D], BF16, tag="xt")
                for nck in range(NCH):
                    xt_ps = xtppool.tile([128, D], BF16, tag="xtp")
                    for dc in range(DC):
                        x_chunk = x_sb[:, dc * N + nck * 128: dc * N + (nck + 1) * 128]
                        nc.tensor.matmul(
                            fT_ps[:, nck * K:(nck + 1) * K],
                            lhsT=x_chunk,
                            rhs=wt_sb[:, dc * K:(dc + 1) * K],
                            start=(dc == 0),
                            stop=(dc == DC - 1),
                        )
                        nc.tensor.transpose(
                            xt_ps[:, dc * 128:(dc + 1) * 128], x_chunk, id_sb[:]
                        )
                    # copy xT chunk PSUM -> SBUF (3 DVE : 5 ACT split)
                    dst = xt_sb[:, nck * D:(nck + 1) * D]
                    if nck % 8 < 3:
                        nc.vector.tensor_copy(dst, xt_ps[:])
                    else:
                        nc.scalar.copy(dst, xt_ps[:])

                # --- softmax over h (partitions hold n = h*32 + w) ---
                eT_sb = spool.tile([128, NCH * K], BF16, tag="e")
                nc.scalar.activation(
                    eT_sb[:], fT_ps[:], mybir.ActivationFunctionType.Exp
                )
                sT_ps = sppool.tile([128, K], F32, tag="sT")
                for nck in range(NCH):
                    nc.tensor.matmul(
                        sT_ps[:],
                        lhsT=hm_sb[:],
                        rhs=eT_sb[:, nck * K:(nck + 1) * K],
                        start=(nck == 0),
                        stop=(nck == NCH - 1),
                    )
                r_tile = spool.tile([128, K], F32, tag="rtile")
                nc.vector.reciprocal(r_tile[:], sT_ps[:])
                aT_sb = spool.tile([128, NCH * K], BF16, tag="a")
                nc.vector.tensor_mul(
                    aT_sb.rearrange("p (c k) -> p c k", c=NCH, k=K),
                    eT_sb.rearrange("p (c k) -> p c k", c=NCH, k=K),
                    r_tile.unsqueeze(1).broadcast_to([128, NCH, K]),
                )

                # --- mm2: V^T[d, k] accumulated over n-chunks ---
                v_ps = vpool.tile([128, DC * K], F32, tag="v")
                for dc in range(DC):
                    for nck in range(NCH):
                        nc.tensor.matmul(
                            v_ps[:, dc * K:(dc + 1) * K],
                            lhsT=xt_sb[:, nck * D + dc * 128: nck * D + (dc + 1) * 128],
                            rhs=aT_sb[:, nck * K:(nck + 1) * K],
                            start=(nck == 0),
                            stop=(nck == NCH - 1),
                        )

                # --- per-sample epilogue part: V - 32*c, squared k-sums ---
                vc = vc_all[:, s * DC * K:(s + 1) * DC * K]
                nc.vector.tensor_sub(vc, v_ps[:], ct_sb[:])
                sq = epool.tile([128, DC * K], F32, tag="sq")
                nc.vector.tensor_mul(sq[:], vc, vc)
                nc.vector.reduce_sum(
                    nsq_all[:, s * DC:(s + 1) * DC],
                    sq.rearrange("p (c k) -> p c k", c=DC, k=K),
                    axis=mybir.AxisListType.X,
                )

            # --- batched normalization tail ---
            rn_all = cpool.tile([128, S * DC], F32)
            nc.vector.reciprocal(rn_all[:], nsq_all[:])
            rs_all = cpool.tile([128, S * DC], F32)
            nc.scalar.sqrt(rs_all[:], rn_all[:])
            y_view = y_all.rearrange("p (c k s) -> p c k s", c=DC, k=K, s=S)
            for s in range(S):
                nc.vector.tensor_mul(
                    y_view[:, :, :, s],
                    vc_all.rearrange("p (s c k) -> p s c k", s=S, c=DC, k=K)[
                        :, s
                    ],
                    rs_all.rearrange("p (s c) -> p s c", s=S, c=DC)[
                        :, s
                    ].unsqueeze(2).broadcast_to([128, DC, K]),
                )

            # --- output: one contiguous DMA per d-chunk ---
            y_out = y.rearrange("(c p) k s -> c p (k s)", c=DC, p=128)
            for dc in range(DC):
                nc.sync.dma_start(
                    y_out[dc], y_all[:, dc * K * S:(dc + 1) * K * S]
                )

    nc.compile()
    return nc


def _get_program():
    if "nc" not in _CACHE:
        _CACHE["nc"] = _build_program()
    return _CACHE["nc"]


def _host_inputs(w: np.ndarray, c: np.ndarray):
    wt_host = np.ascontiguousarray(
        w.T.reshape(DC, 128, K).transpose(1, 0, 2).reshape(128, DC * K)
    ).astype(ml_dtypes.bfloat16)
    ct_host = np.ascontiguousarray(
        (32.0 * c).T.reshape(DC, 128, K).transpose(1, 0, 2).reshape(128, DC * K)
    ).astype(np.float32)
    id_host = np.eye(128, dtype=ml_dtypes.bfloat16)
    hm_host = np.zeros((128, 4 * W), dtype=ml_dtypes.bfloat16)
    for n in range(128):
        for g in range(4):
            hm_host[n, g * W + n % W] = 1.0
    return wt_host, ct_host, id_host, hm_host


def kernel(x: np.ndarray, w: np.ndarray, c: np.ndarray) -> np.ndarray:
    nc = _get_program()

    xs_full = np.ascontiguousarray(x.reshape(B, D, N), dtype=np.float32)
    wt_host, ct_host, id_host, hm_host = _host_inputs(w, c)

    in_maps = []
    for m in range(M):
        in_maps.append(
            {
                "xs": xs_full[m * S:(m + 1) * S],
                "wt": wt_host,
                "ct": ct_host,
                "id128": id_host,
                "hmask": hm_host,
            }
        )

    res = run_bass_kernel_spmd(nc, in_maps, core_ids=list(range(M)))
    y = np.concatenate([res.results[m]["y"] for m in range(M)], axis=2)
    return y.astype(np.float32)


if __name__ == "__main__":
    rng = np.random.default_rng(0)
    x = rng.standard_normal((B, D, H, W), dtype=np.float32)
    w = (rng.standard_normal((K, D)) * 0.05).astype(np.float32)
    c = rng.standard_normal((K, D), dtype=np.float32)
    out = kernel(x=x, w=w, c=c)
    print(out.shape, out.dtype)
